# revision 53
# baseline (speedup 1.0000x reference)
"""Self-contained Trainium2 Bass kernel for MBert self-attention.

Problem (hardcoded): B=4, T=2048, C=768, H=12 heads, D=64.
  q = X @ Wq.T + bq ; k = X @ Wk.T + bk ; v = X @ Wv.T + bv   (per batch)
  scores = q k^T / sqrt(D) + mask_bias ; probs = softmax(scores)
  out = probs @ v                                              (per head)

Sharding over 8 NeuronCores: data-parallel on B (4) x tensor-parallel on
heads (12 -> two groups of 6).  Core c handles batch c//2 and heads
6*(c%2) .. 6*(c%2)+5.  Each core computes its full [T, 384] output slice
locally; host concatenates (no device collectives needed).

Key structure (all matmuls bf16 inputs, fp32 PSUM accumulate):
  - Host pre-transposes X and the W slices (bf16): no PE transposes at
    all.  The K bias is dropped -- softmax((q+bq).(k+bk)) ==
    softmax((q+bq).k) since (q+bq).bk is constant over k.  The V bias
    (a constant per output column) is added on the host.
  - The attention mask enters as the exp bias: scores^T tiles are
    [k partitions, q free] and the additive bias -10000*(1-m_k) is
    per-k == per-partition, exactly the ScalarE activation bias shape.
  - Attention per head pair, per 512-q group, per 128-k chunk: S^T in
    PSUM [128, 2*512], exp on ScalarE (bias fused) -> pT bf16; AV runs
    "flipped": ctx[q, d] with q on partitions and d on the free dim,
    accumulating over the 16 k chunks -- half the PE rows of the ctx^T
    layout.  Each (group, head) owns one PSUM bank [128, 4, 65] holding
    4 q-subtile contexts plus the softmax denominators (V ones-column);
    the 128 matmuls of a group form ONE accumulation group per bank
    (single start zeroes the bank, single stop) per the PSUM zero-region
    rule.
  - ScalarE does exp only -- it is the 199us roofline of this problem
    (25.2M exps/core at 0.83ns/row); DVE does the PSUM->SBUF moves and
    the reciprocal-normalize epilogue; AV is emitted two jobs behind
    S/exp so every semaphore it waits on is pre-satisfied and the exp
    stream stays back-to-back.
  - Projections stream into the ACT-paced attention: pair 0's K/V are
    produced per k-chunk just ahead of its first group's sweep (chasing
    the X DMA pieces); the rest trickle in as deadline-scheduled
    background PE work between jobs.
  - PSUM (8 banks): S^T 2x2, attention accumulators 3x1, projections 1.
"""

from collections import deque

import numpy as np

B, T, C = 4, 2048, 768
H, D = 12, 64
NCORES = 8
HLOC = 6              # heads per core
O = HLOC * D          # 384 output cols per core
NPAIR = HLOC // 2     # 3 head pairs == 3 otiles of 128
CCH = C // 128        # 6 contraction chunks for projections
TT = T // 128         # 16 t tiles
QG = 512              # q-group width
NG = T // QG          # 4 q groups
KCH = T // 128        # 16 k chunks
XPC = 8               # x DMA pieces
XW = T // XPC         # 256 t per x piece
PJW = 256             # startup projection psum width

_CACHE = {}


def _build_nc():
    if "nc" in _CACHE:
        return _CACHE["nc"]

    from contextlib import ExitStack

    import concourse.bass as bass
    import concourse.tile as tile
    from concourse import bacc, mybir

    f32 = mybir.dt.float32
    bf16 = mybir.dt.bfloat16
    EXP = mybir.ActivationFunctionType.Exp

    nc = bacc.Bacc("TRN2", target_bir_lowering=False, debug=False,
                   num_devices=NCORES)

    # host-pretransposed inputs
    x_d = nc.dram_tensor("xt", [C, T], bf16, kind="ExternalInput").ap()
    w_d = {nm: nc.dram_tensor(f"wt{nm}", [C, O], bf16,
                              kind="ExternalInput").ap()
           for nm in ("q", "k", "v")}
    bq_d = nc.dram_tensor("bq", [128, O // 128], f32,
                          kind="ExternalInput").ap()
    lnf_d = nc.dram_tensor("lnf", [128, KCH], f32,
                           kind="ExternalInput").ap()
    o_d = nc.dram_tensor("out", [T, O], bf16,
                         kind="ExternalOutput").ap()

    with tile.TileContext(nc) as tc, ExitStack() as ctx:
        # ---------------- SBUF pools ----------------
        const = ctx.enter_context(tc.tile_pool(name="const", bufs=1))
        xT_pool = ctx.enter_context(tc.tile_pool(name="xT", bufs=1))
        wT_pool = ctx.enter_context(tc.tile_pool(name="wT", bufs=1))
        qkT_pool = ctx.enter_context(tc.tile_pool(name="qkT", bufs=1))
        v_pool = ctx.enter_context(tc.tile_pool(name="v", bufs=1))
        ost_pool = ctx.enter_context(tc.tile_pool(name="ostage", bufs=1))
        pT_pool = ctx.enter_context(tc.tile_pool(name="pT", bufs=4))
        rcp_pool = ctx.enter_context(tc.tile_pool(name="rcp", bufs=4))
        # ---------------- PSUM pools (8 banks total) ----------------
        pst_pool = ctx.enter_context(
            tc.tile_pool(name="pst", bufs=2, space="PSUM"))     # 2x2 banks
        av_pool = ctx.enter_context(
            tc.tile_pool(name="av", bufs=3, space="PSUM"))      # 3x1 bank
        proj_pool = ctx.enter_context(
            tc.tile_pool(name="proj", bufs=1, space="PSUM"))    # 1 bank

        bq_t = const.tile([128, O // 128], f32)     # q bias [o%128, o//128]
        lnf_t = const.tile([128, KCH], f32)         # mask bias [k%128, .//]

        xT = xT_pool.tile([128, CCH, T], bf16)                  # X^T [c, t]
        wT = {nm: wT_pool.tile([128, CCH, O], bf16, name=f"wT_{nm}")
              for nm in ("q", "k", "v")}                        # W^T [c, o]
        qT = qkT_pool.tile([128, NPAIR, T], bf16, name="qT")    # Q^T [o, t]
        kT = qkT_pool.tile([128, NPAIR, T], bf16, name="kT")    # K^T [o, t]
        v_sb = v_pool.tile([128, KCH, HLOC, D + 1], bf16)       # V|1 [k,h,d]
        ostage = ost_pool.tile([128, TT, O], bf16)

        nc.vector.memset(v_sb[:, :, :, D], 1.0)

        # ---------------- input DMAs ----------------
        # order tuned for startup: pair-0 W slices + first x pieces first;
        # lnf/bq arrive host-preshaped ([128, c]: 128 fat descriptors)
        def dma_x(xp):
            nc.sync.dma_start(
                xT[:, :, XW * xp:XW * (xp + 1)],
                x_d[:, XW * xp:XW * (xp + 1)].rearrange(
                    "(cc p) t -> p cc t", p=128))

        def dma_w(nm, o0, o1):
            nc.sync.dma_start(
                wT[nm][:, :, o0:o1],
                w_d[nm][:, o0:o1].rearrange("(cc p) o -> p cc o", p=128))

        dma_x(0)
        dma_w("q", 0, 128)
        dma_w("k", 0, 128)
        nc.sync.dma_start(bq_t[:], bq_d)
        nc.sync.dma_start(lnf_t[:], lnf_d)
        dma_x(1)
        dma_w("v", 0, 128)
        dma_x(2)
        dma_x(3)
        dma_w("k", 128, O)
        dma_w("v", 128, O)
        dma_w("q", 128, O)
        for xp in range(4, XPC):
            dma_x(xp)

        # ---------------- projection emitters ----------------
        def emit_k(p, t0, tw=128, ps=None):
            """kT[:, p, t0:t0+tw] (no bias -- bk cancels in softmax)."""
            ps = ps if ps is not None else proj_pool.tile(
                [128, tw], f32, name="pj", tag="pj")[:]
            for ci in range(CCH):
                nc.tensor.matmul(ps,
                                 lhsT=wT["k"][:, ci, 128 * p:128 * (p + 1)],
                                 rhs=xT[:, ci, t0:t0 + tw],
                                 start=(ci == 0), stop=(ci == CCH - 1))
            nc.vector.tensor_copy(kT[:, p, t0:t0 + tw], ps)

        def emit_q(p, t0, tw=128, ps=None):
            ps = ps if ps is not None else proj_pool.tile(
                [128, tw], f32, name="pj", tag="pj")[:]
            for ci in range(CCH):
                nc.tensor.matmul(ps,
                                 lhsT=wT["q"][:, ci, 128 * p:128 * (p + 1)],
                                 rhs=xT[:, ci, t0:t0 + tw],
                                 start=(ci == 0), stop=(ci == CCH - 1))
            nc.vector.tensor_scalar_add(qT[:, p, t0:t0 + tw], ps,
                                        bq_t[:, p:p + 1])

        def emit_v(p, i, nch=1, ps=None):
            """v_sb[:, i:i+nch, 2p:2p+2, 0:64] (raw V -- bias on host)."""
            o0 = 128 * p
            ps = ps if ps is not None else proj_pool.tile(
                [128, 128 * nch], f32, name="pj", tag="pj")[:]
            for t in range(nch):
                for ci in range(CCH):
                    nc.tensor.matmul(
                        ps[:, 128 * t:128 * (t + 1)],
                        lhsT=xT[:, ci, 128 * (i + t):128 * (i + t + 1)],
                        rhs=wT["v"][:, ci, o0:o0 + 128],
                        start=(ci == 0), stop=(ci == CCH - 1))
            nc.vector.tensor_copy(
                v_sb[:, i:i + nch, 2 * p:2 * p + 2, 0:D],
                ps.rearrange("p (t h d) -> p t h d", t=nch, h=2))

        # ---------------- attention emitters ----------------
        pT_all = {}
        av_ps = {}

        def emit_st(job):
            p, g, i = job
            q0 = QG * g
            if i == 0:
                if (p, g) == (0, 0):
                    # interleave banks: per head [128, NG, 66] = group-0
                    # ctx + den (col 64) + group-1 den (col 65)
                    for h in (0, 1):
                        av_ps[(0, 0, h)] = av_pool.tile(
                            [128, NG, D + 2], f32, name=f"av{h}", tag="av")
                elif (p, g) == (0, 1):
                    # group-1 ctx fills the third bank exactly
                    av_ps[(0, 1, "c")] = av_pool.tile(
                        [128, 2, NG, D], f32, name="avc", tag="av")
                else:
                    for h in (0, 1):
                        av_ps[(p, g, h)] = av_pool.tile(
                            [128, NG, D + 1], f32, name=f"av{h}", tag="av")
            st = pst_pool.tile([128, 2 * QG], f32, name="st", tag="st")
            nc.tensor.matmul(st[:, 0:QG],
                             lhsT=kT[0:64, p, 128 * i:128 * (i + 1)],
                             rhs=qT[0:64, p, q0:q0 + QG])
            nc.tensor.matmul(st[:, QG:2 * QG],
                             lhsT=kT[64:128, p, 128 * i:128 * (i + 1)],
                             rhs=qT[64:128, p, q0:q0 + QG])
            pT = pT_pool.tile([128, 2 * QG], bf16, name="pT", tag="pT")
            nc.scalar.activation(pT[:], st[:], EXP, scale=0.125,
                                 bias=lnf_t[:, i:i + 1])
            pT_all[job] = pT

        def emit_av(job):
            """ctx[q,d] += pT.T V ; den[q] += pT.T 1.  All matmuls into
            a bank form ONE psum accumulation group: only the very first
            has start (zeroing the bank), only the very last has stop.
            Pair-0 groups 0/1 are interleaved: group 0's per-head banks
            also hold group 1's denominators (their stop), and group 1's
            ctx fills the third bank."""
            p, g, i = job
            pT = pT_all.pop(job)
            last = (i == KCH - 1)
            if (p, g) == (0, 1):
                c2 = av_ps[(0, 1, "c")]
                for h in (0, 1):
                    acch = av_ps[(0, 0, h)]
                    for qs in range(NG):
                        lhsT = pT[:, QG * h + 128 * qs:
                                  QG * h + 128 * (qs + 1)]
                        nc.tensor.matmul(
                            c2[:, h, qs, :], lhsT=lhsT,
                            rhs=v_sb[:, i, h, 0:D],
                            start=(i == 0 and qs == 0 and h == 0),
                            stop=(last and qs == NG - 1 and h == 1))
                        nc.tensor.matmul(
                            acch[:, qs, D + 1:D + 2], lhsT=lhsT,
                            rhs=v_sb[:, i, h, D:D + 1],
                            start=False,
                            stop=(last and qs == NG - 1))
                return
            ga = (p, g) == (0, 0)
            for h in (0, 1):
                hh = 2 * p + h
                acc = av_ps[(p, g, h)]
                for qs in range(NG):
                    lhsT = pT[:, QG * h + 128 * qs:QG * h + 128 * (qs + 1)]
                    nc.tensor.matmul(acc[:, qs, 0:D], lhsT=lhsT,
                                     rhs=v_sb[:, i, hh, 0:D],
                                     start=(i == 0 and qs == 0), stop=False)
                    nc.tensor.matmul(acc[:, qs, D:D + 1], lhsT=lhsT,
                                     rhs=v_sb[:, i, hh, D:D + 1],
                                     start=False,
                                     stop=(not ga and last
                                           and qs == NG - 1))

        def emit_epilogue_pair0():
            """Joint epilogue for the interleaved pair-0 groups 0/1.
            The group-0 banks are read first so group 2's accumulation
            (which reuses them) is not held up by the full drain."""
            c2 = av_ps.pop((0, 1, "c"))
            rcps = []
            for h in (0, 1):
                acc = av_ps.pop((0, 0, h))
                rcp = rcp_pool.tile([128, 2, NG], f32, name="rcp0",
                                    tag="rcp")
                nc.vector.reciprocal(rcp[:, 0], acc[:, :, D])
                nc.vector.reciprocal(rcp[:, 1], acc[:, :, D + 1])
                rcps.append(rcp)
                o0 = D * h
                for qs in range(NG):
                    nc.vector.tensor_scalar_mul(
                        ostage[:, qs, o0:o0 + D], acc[:, qs, 0:D],
                        rcp[:, 0, qs:qs + 1])
            for h in (0, 1):
                o0 = D * h
                for qs in range(NG):
                    nc.vector.tensor_scalar_mul(
                        ostage[:, NG + qs, o0:o0 + D], c2[:, h, qs, :],
                        rcps[h][:, 1, qs:qs + 1])

        def emit_epilogue(p, g, final=False):
            """out = ctx/den on DVE (V bias is added on the host).  The
            final group splits the normalize across DVE and the now-idle
            ScalarE (Copy activation scaled by the reciprocal) and DMAs
            each t tile out as soon as both heads land."""
            if not final:
                for h in (0, 1):
                    acc = av_ps.pop((p, g, h))
                    rcp = rcp_pool.tile([128, NG], f32, name="rcp",
                                        tag="rcp")
                    nc.vector.reciprocal(rcp[:], acc[:, :, D])
                    o0 = D * (2 * p + h)
                    for qs in range(NG):
                        it = NG * g + qs
                        nc.vector.tensor_scalar_mul(
                            ostage[:, it, o0:o0 + D], acc[:, qs, 0:D],
                            rcp[:, qs:qs + 1])
                return
            COPY = mybir.ActivationFunctionType.Copy
            acc0 = av_ps.pop((p, g, 0))
            acc1 = av_ps.pop((p, g, 1))
            rcp = rcp_pool.tile([128, 2, NG], f32, name="rcpf", tag="rcp")
            nc.vector.reciprocal(rcp[:, 0], acc0[:, :, D])
            nc.vector.reciprocal(rcp[:, 1], acc1[:, :, D])
            o0, o1 = D * 2 * p, D * (2 * p + 1)
            for qs in range(NG):
                it = NG * g + qs
                nc.vector.tensor_scalar_mul(
                    ostage[:, it, o0:o0 + D], acc0[:, qs, 0:D],
                    rcp[:, 0, qs:qs + 1])
                nc.scalar.activation(
                    ostage[:, it, o1:o1 + D], acc1[:, qs, 0:D], COPY,
                    scale=rcp[:, 1, qs:qs + 1])
                emit_out_dma(it)

        def emit_out_dma(it):
            nc.sync.dma_start(o_d[128 * it:128 * (it + 1), :],
                              ostage[:, it, :])

        # ---------------- schedule ----------------
        # pair-0 groups 0/1 interleave (group 0 two chunks ahead) so the
        # K/V chase costs half per job; groups 2/3 and pairs 1/2 are
        # sequential as before (same job indices from 32 on).
        jobs = [(0, 0, 0), (0, 0, 1), (0, 0, 2), (0, 0, 3)]
        for k in range(KCH - 4):
            jobs += [(0, 1, k), (0, 0, k + 4)]
        jobs += [(0, 1, k2) for k2 in range(KCH - 4, KCH)]
        jobs += [(0, g, i) for g in (2, 3) for i in range(KCH)]
        jobs += [(p, g, i) for p in (1, 2) for g in range(NG)
                 for i in range(KCH)]

        # Background PE work (pair-0 k-chunk chase is inline; all other
        # projections live in one deadline-guarded queue popped between
        # jobs at a steady ~0.8/job).
        bgq = []

        def add_bg(dl, fn):
            bgq.append((dl, len(bgq), fn))

        for g in (2, 3):                   # pair-0 groups 2,3 qT
            for s in range(4):
                add_bg(16 * g - 14 + 2 * s,
                       lambda t0=QG * g + 128 * s: emit_q(0, t0))
        for c in range(KCH):               # pair-1 K/V
            add_bg(28 + c, lambda c=c: emit_k(1, 128 * c))
            add_bg(29 + c, lambda c=c: emit_v(1, c))
        for s in range(4):                 # pair-1 group-0 qT
            add_bg(46 + 2 * s, lambda t0=128 * s: emit_q(1, t0))
        for g in (1, 2, 3):                # pair-1 groups 1..3 qT
            for s in range(4):
                add_bg(64 + 16 * g - 8 + 2 * s,
                       lambda t0=QG * g + 128 * s: emit_q(1, t0))
        for c in range(KCH):               # pair-2 K/V
            add_bg(92 + c, lambda c=c: emit_k(2, 128 * c))
            add_bg(93 + c, lambda c=c: emit_v(2, c))
        for s in range(4):                 # pair-2 group-0 qT
            add_bg(110 + 2 * s, lambda t0=128 * s: emit_q(2, t0))
        for g in (1, 2, 3):                # pair-2 groups 1..3 qT
            for s in range(4):
                add_bg(128 + 16 * g - 8 + 2 * s,
                       lambda t0=QG * g + 128 * s: emit_q(2, t0))
        bgq.sort()
        bg = deque(fn for _, _, fn in bgq)
        bg_dl = deque(dl for dl, _, _ in bgq)

        def emit_post(n):
            p, g, i = jobs[n]
            if (p, g) == (0, 0):           # chase the x DMA pieces
                if i % 2 == 0 and i + 3 < KCH:
                    emit_k(0, 128 * (i + 2), 256)
                elif i % 2 == 1 and i + 2 < KCH:
                    emit_v(0, i + 1, nch=2)
            if n < 16:
                return
            if bg and (bg_dl[0] <= n + 6 or n % 5 != 4):
                bg_dl.popleft()
                bg.popleft()()

        # startup: qT group 0 + kT chunk 0 ahead of the first S; V
        # chunks 0-1 right after (needed by the first AV, two jobs in)
        emit_q(0, 0, PJW, ps=pst_pool.tile([128, PJW], f32, name="sq0",
                                           tag="st")[:])
        emit_q(0, PJW, PJW, ps=pst_pool.tile([128, PJW], f32, name="sq1",
                                             tag="st")[:])
        emit_k(0, 0)
        emit_st(jobs[0])
        emit_k(0, 128)
        emit_st(jobs[1])
        emit_k(0, 256, 256)   # chunks 2,3 (emit_post(0)/(1) never run)
        emit_v(0, 0, nch=2)
        emit_q(0, QG, PJW, ps=pst_pool.tile([128, PJW], f32, name="sq2",
                                            tag="st")[:])
        emit_q(0, QG + PJW, PJW,
               ps=pst_pool.tile([128, PJW], f32, name="sq3", tag="st")[:])
        # AV runs two jobs behind S/exp so its waits are pre-satisfied
        for n in range(2, len(jobs)):
            emit_st(jobs[n])
            if n == 2:         # V chunks 2,3: after S(A2), before AV(A2)
                emit_v(0, 2, nch=2)
            if n >= 2:
                done = jobs[n - 2]
                emit_av(done)
                if done[2] == KCH - 1 and done[0:2] != (0, 0):
                    p, g = done[0], done[1]
                    if (p, g) == (0, 1):
                        emit_epilogue_pair0()
                    else:
                        emit_epilogue(p, g)
                        if p == NPAIR - 1:
                            for qs in range(NG):
                                emit_out_dma(NG * g + qs)
            emit_post(n)
        emit_av(jobs[-2])
        emit_av(jobs[-1])
        emit_epilogue(jobs[-1][0], jobs[-1][1], final=True)

    nc.compile()
    _CACHE["nc"] = nc
    return nc


def _in_maps(inputs):
    import ml_dtypes
    bf16 = ml_dtypes.bfloat16

    hs = np.asarray(inputs["hidden_states"], dtype=np.float32)
    mask = np.asarray(inputs["attention_mask"], dtype=np.float32)
    W = {nm: np.asarray(inputs["W" + nm], dtype=np.float32)
         for nm in ("q", "k", "v")}
    bq = np.asarray(inputs["bq"], dtype=np.float32)
    lnf = (mask - 1.0) * 10000.0
    xts = [np.ascontiguousarray(hs[b].T).astype(bf16) for b in range(B)]
    # device layouts: lnf [k%128, k//128], bq [o%128, o//128]
    lnfs = [np.ascontiguousarray(lnf[b].reshape(KCH, 128).T)
            for b in range(B)]
    maps = []
    for c in range(NCORES):
        b, hh = divmod(c, 2)
        o0 = hh * O
        m = {"xt": xts[b], "lnf": lnfs[b]}
        for nm in ("q", "k", "v"):
            m["wt" + nm] = np.ascontiguousarray(
                W[nm][o0:o0 + O].T).astype(bf16)
        m["bq"] = np.ascontiguousarray(
            bq[o0:o0 + O].reshape(O // 128, 128).T)
        maps.append(m)
    return maps


def run_on_cores(inputs, **spmd_kwargs):
    """Build (cached), run on the 8 NeuronCores, return BassKernelResults."""
    from concourse import bass_utils
    nc = _build_nc()
    return bass_utils.run_bass_kernel_spmd(
        nc, _in_maps(inputs), core_ids=list(range(NCORES)), **spmd_kwargs)


def kernel(**inputs):
    res = run_on_cores(inputs)
    out = np.empty((B, T, C), dtype=np.float32)
    for c in range(NCORES):
        b, hh = divmod(c, 2)
        out[b, :, hh * O:(hh + 1) * O] = \
            res.results[c]["out"].astype(np.float32)
    # V bias is a constant per output column; applied here (exact)
    out += np.asarray(inputs["bv"], dtype=np.float32)[None, None, :]
    return out


# revision 54
# speedup vs baseline: 1.0010x; 1.0010x over previous
"""Self-contained Trainium2 Bass kernel for MBert self-attention.

Problem (hardcoded): B=4, T=2048, C=768, H=12 heads, D=64.
  q = X @ Wq.T + bq ; k = X @ Wk.T + bk ; v = X @ Wv.T + bv   (per batch)
  scores = q k^T / sqrt(D) + mask_bias ; probs = softmax(scores)
  out = probs @ v                                              (per head)

Sharding over 8 NeuronCores: data-parallel on B (4) x tensor-parallel on
heads (12 -> two groups of 6).  Core c handles batch c//2 and heads
6*(c%2) .. 6*(c%2)+5.  Each core computes its full [T, 384] output slice
locally; host concatenates (no device collectives needed).

Key structure (all matmuls bf16 inputs, fp32 PSUM accumulate):
  - Host pre-transposes X and the W slices (bf16): no PE transposes at
    all.  The K bias is dropped -- softmax((q+bq).(k+bk)) ==
    softmax((q+bq).k) since (q+bq).bk is constant over k.  The V bias
    (a constant per output column) is added on the host.
  - The attention mask enters as the exp bias: scores^T tiles are
    [k partitions, q free] and the additive bias -10000*(1-m_k) is
    per-k == per-partition, exactly the ScalarE activation bias shape.
  - Attention per head pair, per 512-q group, per 128-k chunk: S^T in
    PSUM [128, 2*512], exp on ScalarE (bias fused) -> pT bf16; AV runs
    "flipped": ctx[q, d] with q on partitions and d on the free dim,
    accumulating over the 16 k chunks -- half the PE rows of the ctx^T
    layout.  Each (group, head) owns one PSUM bank [128, 4, 65] holding
    4 q-subtile contexts plus the softmax denominators (V ones-column);
    the 128 matmuls of a group form ONE accumulation group per bank
    (single start zeroes the bank, single stop) per the PSUM zero-region
    rule.
  - ScalarE does exp only -- it is the 199us roofline of this problem
    (25.2M exps/core at 0.83ns/row); DVE does the PSUM->SBUF moves and
    the reciprocal-normalize epilogue; AV is emitted two jobs behind
    S/exp so every semaphore it waits on is pre-satisfied and the exp
    stream stays back-to-back.
  - Projections stream into the ACT-paced attention: pair 0's K/V are
    produced per k-chunk just ahead of its first group's sweep (chasing
    the X DMA pieces); the rest trickle in as deadline-scheduled
    background PE work between jobs.
  - PSUM (8 banks): S^T 2x2, attention accumulators 3x1, projections 1.
"""

from collections import deque

import numpy as np

B, T, C = 4, 2048, 768
H, D = 12, 64
NCORES = 8
HLOC = 6              # heads per core
O = HLOC * D          # 384 output cols per core
NPAIR = HLOC // 2     # 3 head pairs == 3 otiles of 128
CCH = C // 128        # 6 contraction chunks for projections
TT = T // 128         # 16 t tiles
QG = 512              # q-group width
NG = T // QG          # 4 q groups
KCH = T // 128        # 16 k chunks
XPC = 8               # x DMA pieces
XW = T // XPC         # 256 t per x piece
PJW = 256             # startup projection psum width

_CACHE = {}


def _build_nc():
    if "nc" in _CACHE:
        return _CACHE["nc"]

    from contextlib import ExitStack

    import concourse.bass as bass
    import concourse.tile as tile
    from concourse import bacc, mybir

    f32 = mybir.dt.float32
    bf16 = mybir.dt.bfloat16
    EXP = mybir.ActivationFunctionType.Exp

    nc = bacc.Bacc("TRN2", target_bir_lowering=False, debug=False,
                   num_devices=NCORES)

    # host-pretransposed inputs
    x_d = nc.dram_tensor("xt", [C, T], bf16, kind="ExternalInput").ap()
    w_d = {nm: nc.dram_tensor(f"wt{nm}", [C, O], bf16,
                              kind="ExternalInput").ap()
           for nm in ("q", "k", "v")}
    bq_d = nc.dram_tensor("bq", [128, O // 128], f32,
                          kind="ExternalInput").ap()
    lnf_d = nc.dram_tensor("lnf", [128, KCH], f32,
                           kind="ExternalInput").ap()
    o_d = nc.dram_tensor("out", [T, O], bf16,
                         kind="ExternalOutput").ap()

    with tile.TileContext(nc) as tc, ExitStack() as ctx:
        # ---------------- SBUF pools ----------------
        const = ctx.enter_context(tc.tile_pool(name="const", bufs=1))
        xT_pool = ctx.enter_context(tc.tile_pool(name="xT", bufs=1))
        wT_pool = ctx.enter_context(tc.tile_pool(name="wT", bufs=1))
        qkT_pool = ctx.enter_context(tc.tile_pool(name="qkT", bufs=1))
        v_pool = ctx.enter_context(tc.tile_pool(name="v", bufs=1))
        ost_pool = ctx.enter_context(tc.tile_pool(name="ostage", bufs=1))
        pT_pool = ctx.enter_context(tc.tile_pool(name="pT", bufs=5))
        rcp_pool = ctx.enter_context(tc.tile_pool(name="rcp", bufs=4))
        # ---------------- PSUM pools (8 banks total) ----------------
        pst_pool = ctx.enter_context(
            tc.tile_pool(name="pst", bufs=2, space="PSUM"))     # 2x2 banks
        av_pool = ctx.enter_context(
            tc.tile_pool(name="av", bufs=3, space="PSUM"))      # 3x1 bank
        proj_pool = ctx.enter_context(
            tc.tile_pool(name="proj", bufs=1, space="PSUM"))    # 1 bank

        bq_t = const.tile([128, O // 128], f32)     # q bias [o%128, o//128]
        lnf_t = const.tile([128, KCH], f32)         # mask bias [k%128, .//]

        xT = xT_pool.tile([128, CCH, T], bf16)                  # X^T [c, t]
        wT = {nm: wT_pool.tile([128, CCH, O], bf16, name=f"wT_{nm}")
              for nm in ("q", "k", "v")}                        # W^T [c, o]
        qT = qkT_pool.tile([128, NPAIR, T], bf16, name="qT")    # Q^T [o, t]
        kT = qkT_pool.tile([128, NPAIR, T], bf16, name="kT")    # K^T [o, t]
        v_sb = v_pool.tile([128, KCH, HLOC, D + 1], bf16)       # V|1 [k,h,d]
        ostage = ost_pool.tile([128, TT, O], bf16)

        nc.vector.memset(v_sb[:, :, :, D], 1.0)

        # ---------------- input DMAs ----------------
        # order tuned for startup: pair-0 W slices + first x pieces first;
        # lnf/bq arrive host-preshaped ([128, c]: 128 fat descriptors)
        def dma_x(xp):
            nc.sync.dma_start(
                xT[:, :, XW * xp:XW * (xp + 1)],
                x_d[:, XW * xp:XW * (xp + 1)].rearrange(
                    "(cc p) t -> p cc t", p=128))

        def dma_w(nm, o0, o1):
            nc.sync.dma_start(
                wT[nm][:, :, o0:o1],
                w_d[nm][:, o0:o1].rearrange("(cc p) o -> p cc o", p=128))

        dma_x(0)
        dma_w("q", 0, 128)
        dma_w("k", 0, 128)
        nc.sync.dma_start(bq_t[:], bq_d)
        nc.sync.dma_start(lnf_t[:], lnf_d)
        dma_x(1)
        dma_w("v", 0, 128)
        dma_x(2)
        dma_x(3)
        dma_w("k", 128, O)
        dma_w("v", 128, O)
        dma_w("q", 128, O)
        for xp in range(4, XPC):
            dma_x(xp)

        # ---------------- projection emitters ----------------
        def emit_k(p, t0, tw=128, ps=None):
            """kT[:, p, t0:t0+tw] (no bias -- bk cancels in softmax)."""
            ps = ps if ps is not None else proj_pool.tile(
                [128, tw], f32, name="pj", tag="pj")[:]
            for ci in range(CCH):
                nc.tensor.matmul(ps,
                                 lhsT=wT["k"][:, ci, 128 * p:128 * (p + 1)],
                                 rhs=xT[:, ci, t0:t0 + tw],
                                 start=(ci == 0), stop=(ci == CCH - 1))
            nc.vector.tensor_copy(kT[:, p, t0:t0 + tw], ps)

        def emit_q(p, t0, tw=128, ps=None):
            ps = ps if ps is not None else proj_pool.tile(
                [128, tw], f32, name="pj", tag="pj")[:]
            for ci in range(CCH):
                nc.tensor.matmul(ps,
                                 lhsT=wT["q"][:, ci, 128 * p:128 * (p + 1)],
                                 rhs=xT[:, ci, t0:t0 + tw],
                                 start=(ci == 0), stop=(ci == CCH - 1))
            nc.vector.tensor_scalar_add(qT[:, p, t0:t0 + tw], ps,
                                        bq_t[:, p:p + 1])

        def emit_v(p, i, nch=1, ps=None):
            """v_sb[:, i:i+nch, 2p:2p+2, 0:64] (raw V -- bias on host)."""
            o0 = 128 * p
            ps = ps if ps is not None else proj_pool.tile(
                [128, 128 * nch], f32, name="pj", tag="pj")[:]
            for t in range(nch):
                for ci in range(CCH):
                    nc.tensor.matmul(
                        ps[:, 128 * t:128 * (t + 1)],
                        lhsT=xT[:, ci, 128 * (i + t):128 * (i + t + 1)],
                        rhs=wT["v"][:, ci, o0:o0 + 128],
                        start=(ci == 0), stop=(ci == CCH - 1))
            nc.vector.tensor_copy(
                v_sb[:, i:i + nch, 2 * p:2 * p + 2, 0:D],
                ps.rearrange("p (t h d) -> p t h d", t=nch, h=2))

        # ---------------- attention emitters ----------------
        pT_all = {}
        av_ps = {}

        def emit_st(job):
            p, g, i = job
            q0 = QG * g
            if i == 0:
                if (p, g) == (0, 0):
                    # interleave banks: per head [128, NG, 66] = group-0
                    # ctx + den (col 64) + group-1 den (col 65)
                    for h in (0, 1):
                        av_ps[(0, 0, h)] = av_pool.tile(
                            [128, NG, D + 2], f32, name=f"av{h}", tag="av")
                elif (p, g) == (0, 1):
                    # group-1 ctx fills the third bank exactly
                    av_ps[(0, 1, "c")] = av_pool.tile(
                        [128, 2, NG, D], f32, name="avc", tag="av")
                else:
                    for h in (0, 1):
                        av_ps[(p, g, h)] = av_pool.tile(
                            [128, NG, D + 1], f32, name=f"av{h}", tag="av")
            st = pst_pool.tile([128, 2 * QG], f32, name="st", tag="st")
            nc.tensor.matmul(st[:, 0:QG],
                             lhsT=kT[0:64, p, 128 * i:128 * (i + 1)],
                             rhs=qT[0:64, p, q0:q0 + QG])
            nc.tensor.matmul(st[:, QG:2 * QG],
                             lhsT=kT[64:128, p, 128 * i:128 * (i + 1)],
                             rhs=qT[64:128, p, q0:q0 + QG])
            pT = pT_pool.tile([128, 2 * QG], bf16, name="pT", tag="pT")
            nc.scalar.activation(pT[:], st[:], EXP, scale=0.125,
                                 bias=lnf_t[:, i:i + 1])
            pT_all[job] = pT

        def emit_av(job):
            """ctx[q,d] += pT.T V ; den[q] += pT.T 1.  All matmuls into
            a bank form ONE psum accumulation group: only the very first
            has start (zeroing the bank), only the very last has stop.
            Pair-0 groups 0/1 are interleaved: group 0's per-head banks
            also hold group 1's denominators (their stop), and group 1's
            ctx fills the third bank."""
            p, g, i = job
            pT = pT_all.pop(job)
            last = (i == KCH - 1)
            if (p, g) == (0, 1):
                c2 = av_ps[(0, 1, "c")]
                for h in (0, 1):
                    acch = av_ps[(0, 0, h)]
                    for qs in range(NG):
                        lhsT = pT[:, QG * h + 128 * qs:
                                  QG * h + 128 * (qs + 1)]
                        nc.tensor.matmul(
                            c2[:, h, qs, :], lhsT=lhsT,
                            rhs=v_sb[:, i, h, 0:D],
                            start=(i == 0 and qs == 0 and h == 0),
                            stop=(last and qs == NG - 1 and h == 1))
                        nc.tensor.matmul(
                            acch[:, qs, D + 1:D + 2], lhsT=lhsT,
                            rhs=v_sb[:, i, h, D:D + 1],
                            start=False,
                            stop=(last and qs == NG - 1))
                return
            ga = (p, g) == (0, 0)
            for h in (0, 1):
                hh = 2 * p + h
                acc = av_ps[(p, g, h)]
                for qs in range(NG):
                    lhsT = pT[:, QG * h + 128 * qs:QG * h + 128 * (qs + 1)]
                    nc.tensor.matmul(acc[:, qs, 0:D], lhsT=lhsT,
                                     rhs=v_sb[:, i, hh, 0:D],
                                     start=(i == 0 and qs == 0), stop=False)
                    nc.tensor.matmul(acc[:, qs, D:D + 1], lhsT=lhsT,
                                     rhs=v_sb[:, i, hh, D:D + 1],
                                     start=False,
                                     stop=(not ga and last
                                           and qs == NG - 1))

        def emit_epilogue_pair0():
            """Joint epilogue for the interleaved pair-0 groups 0/1.
            The group-0 banks are read first so group 2's accumulation
            (which reuses them) is not held up by the full drain."""
            c2 = av_ps.pop((0, 1, "c"))
            rcps = []
            for h in (0, 1):
                acc = av_ps.pop((0, 0, h))
                rcp = rcp_pool.tile([128, 2, NG], f32, name="rcp0",
                                    tag="rcp")
                nc.vector.reciprocal(rcp[:, 0], acc[:, :, D])
                nc.vector.reciprocal(rcp[:, 1], acc[:, :, D + 1])
                rcps.append(rcp)
                o0 = D * h
                for qs in range(NG):
                    nc.vector.tensor_scalar_mul(
                        ostage[:, qs, o0:o0 + D], acc[:, qs, 0:D],
                        rcp[:, 0, qs:qs + 1])
            for h in (0, 1):
                o0 = D * h
                for qs in range(NG):
                    nc.vector.tensor_scalar_mul(
                        ostage[:, NG + qs, o0:o0 + D], c2[:, h, qs, :],
                        rcps[h][:, 1, qs:qs + 1])

        def emit_epilogue(p, g, final=False):
            """out = ctx/den on DVE (V bias is added on the host).  The
            final group splits the normalize across DVE and the now-idle
            ScalarE (Copy activation scaled by the reciprocal) and DMAs
            each t tile out as soon as both heads land."""
            if not final:
                for h in (0, 1):
                    acc = av_ps.pop((p, g, h))
                    rcp = rcp_pool.tile([128, NG], f32, name="rcp",
                                        tag="rcp")
                    nc.vector.reciprocal(rcp[:], acc[:, :, D])
                    o0 = D * (2 * p + h)
                    for qs in range(NG):
                        it = NG * g + qs
                        nc.vector.tensor_scalar_mul(
                            ostage[:, it, o0:o0 + D], acc[:, qs, 0:D],
                            rcp[:, qs:qs + 1])
                return
            COPY = mybir.ActivationFunctionType.Copy
            acc0 = av_ps.pop((p, g, 0))
            acc1 = av_ps.pop((p, g, 1))
            rcp = rcp_pool.tile([128, 2, NG], f32, name="rcpf", tag="rcp")
            nc.vector.reciprocal(rcp[:, 0], acc0[:, :, D])
            nc.vector.reciprocal(rcp[:, 1], acc1[:, :, D])
            o0, o1 = D * 2 * p, D * (2 * p + 1)
            for qs in range(NG):
                it = NG * g + qs
                nc.vector.tensor_scalar_mul(
                    ostage[:, it, o0:o0 + D], acc0[:, qs, 0:D],
                    rcp[:, 0, qs:qs + 1])
                nc.scalar.activation(
                    ostage[:, it, o1:o1 + D], acc1[:, qs, 0:D], COPY,
                    scale=rcp[:, 1, qs:qs + 1])
                emit_out_dma(it)

        def emit_out_dma(it):
            nc.sync.dma_start(o_d[128 * it:128 * (it + 1), :],
                              ostage[:, it, :])

        # ---------------- schedule ----------------
        # pair-0 groups 0/1 interleave (group 0 two chunks ahead) so the
        # K/V chase costs half per job; groups 2/3 and pairs 1/2 are
        # sequential as before (same job indices from 32 on).
        jobs = [(0, 0, 0), (0, 0, 1), (0, 0, 2), (0, 0, 3)]
        for k in range(KCH - 4):
            jobs += [(0, 1, k), (0, 0, k + 4)]
        jobs += [(0, 1, k2) for k2 in range(KCH - 4, KCH)]
        jobs += [(0, g, i) for g in (2, 3) for i in range(KCH)]
        jobs += [(p, g, i) for p in (1, 2) for g in range(NG)
                 for i in range(KCH)]

        # Background PE work (pair-0 k-chunk chase is inline; all other
        # projections live in one deadline-guarded queue popped between
        # jobs at a steady ~0.8/job).
        bgq = []

        def add_bg(dl, fn):
            bgq.append((dl, len(bgq), fn))

        for g in (2, 3):                   # pair-0 groups 2,3 qT
            for s in range(4):
                add_bg(16 * g - 14 + 2 * s,
                       lambda t0=QG * g + 128 * s: emit_q(0, t0))
        for c in range(KCH):               # pair-1 K/V
            add_bg(28 + c, lambda c=c: emit_k(1, 128 * c))
            add_bg(29 + c, lambda c=c: emit_v(1, c))
        for s in range(4):                 # pair-1 group-0 qT
            add_bg(46 + 2 * s, lambda t0=128 * s: emit_q(1, t0))
        for g in (1, 2, 3):                # pair-1 groups 1..3 qT
            for s in range(4):
                add_bg(64 + 16 * g - 8 + 2 * s,
                       lambda t0=QG * g + 128 * s: emit_q(1, t0))
        for c in range(KCH):               # pair-2 K/V
            add_bg(92 + c, lambda c=c: emit_k(2, 128 * c))
            add_bg(93 + c, lambda c=c: emit_v(2, c))
        for s in range(4):                 # pair-2 group-0 qT
            add_bg(110 + 2 * s, lambda t0=128 * s: emit_q(2, t0))
        for g in (1, 2, 3):                # pair-2 groups 1..3 qT
            for s in range(4):
                add_bg(128 + 16 * g - 8 + 2 * s,
                       lambda t0=QG * g + 128 * s: emit_q(2, t0))
        bgq.sort()
        bg = deque(fn for _, _, fn in bgq)
        bg_dl = deque(dl for dl, _, _ in bgq)

        def emit_post(n):
            p, g, i = jobs[n]
            if (p, g) == (0, 0):           # chase the x DMA pieces
                if i % 2 == 0 and i + 3 < KCH:
                    emit_k(0, 128 * (i + 2), 256)
                elif i % 2 == 1 and i + 2 < KCH:
                    emit_v(0, i + 1, nch=2)
            if n < 16:
                return
            if bg and (bg_dl[0] <= n + 6 or n % 5 != 4):
                bg_dl.popleft()
                bg.popleft()()

        # startup: qT group 0 + kT chunk 0 ahead of the first S; V
        # chunks 0-1 right after (needed by the first AV, two jobs in)
        emit_q(0, 0, PJW, ps=pst_pool.tile([128, PJW], f32, name="sq0",
                                           tag="st")[:])
        emit_q(0, PJW, PJW, ps=pst_pool.tile([128, PJW], f32, name="sq1",
                                             tag="st")[:])
        emit_k(0, 0)
        emit_st(jobs[0])
        emit_k(0, 128)
        emit_st(jobs[1])
        emit_k(0, 256, 256)   # chunks 2,3 (emit_post(0)/(1) never run)
        emit_v(0, 0, nch=2)
        emit_q(0, QG, PJW, ps=pst_pool.tile([128, PJW], f32, name="sq2",
                                            tag="st")[:])
        emit_q(0, QG + PJW, PJW,
               ps=pst_pool.tile([128, PJW], f32, name="sq3", tag="st")[:])
        # AV runs two jobs behind S/exp so its waits are pre-satisfied
        for n in range(2, len(jobs)):
            emit_st(jobs[n])
            if n == 2:         # V chunks 2,3: after S(A2), before AV(A2)
                emit_v(0, 2, nch=2)
            if n >= 2:
                done = jobs[n - 2]
                emit_av(done)
                if done[2] == KCH - 1 and done[0:2] != (0, 0):
                    p, g = done[0], done[1]
                    if (p, g) == (0, 1):
                        emit_epilogue_pair0()
                    else:
                        emit_epilogue(p, g)
                        if p == NPAIR - 1:
                            for qs in range(NG):
                                emit_out_dma(NG * g + qs)
            emit_post(n)
        emit_av(jobs[-2])
        emit_av(jobs[-1])
        emit_epilogue(jobs[-1][0], jobs[-1][1], final=True)

    nc.compile()
    _CACHE["nc"] = nc
    return nc


def _in_maps(inputs):
    import ml_dtypes
    bf16 = ml_dtypes.bfloat16

    hs = np.asarray(inputs["hidden_states"], dtype=np.float32)
    mask = np.asarray(inputs["attention_mask"], dtype=np.float32)
    W = {nm: np.asarray(inputs["W" + nm], dtype=np.float32)
         for nm in ("q", "k", "v")}
    bq = np.asarray(inputs["bq"], dtype=np.float32)
    lnf = (mask - 1.0) * 10000.0
    xts = [np.ascontiguousarray(hs[b].T).astype(bf16) for b in range(B)]
    # device layouts: lnf [k%128, k//128], bq [o%128, o//128]
    lnfs = [np.ascontiguousarray(lnf[b].reshape(KCH, 128).T)
            for b in range(B)]
    maps = []
    for c in range(NCORES):
        b, hh = divmod(c, 2)
        o0 = hh * O
        m = {"xt": xts[b], "lnf": lnfs[b]}
        for nm in ("q", "k", "v"):
            m["wt" + nm] = np.ascontiguousarray(
                W[nm][o0:o0 + O].T).astype(bf16)
        m["bq"] = np.ascontiguousarray(
            bq[o0:o0 + O].reshape(O // 128, 128).T)
        maps.append(m)
    return maps


def run_on_cores(inputs, **spmd_kwargs):
    """Build (cached), run on the 8 NeuronCores, return BassKernelResults."""
    from concourse import bass_utils
    nc = _build_nc()
    return bass_utils.run_bass_kernel_spmd(
        nc, _in_maps(inputs), core_ids=list(range(NCORES)), **spmd_kwargs)


def kernel(**inputs):
    res = run_on_cores(inputs)
    out = np.empty((B, T, C), dtype=np.float32)
    for c in range(NCORES):
        b, hh = divmod(c, 2)
        out[b, :, hh * O:(hh + 1) * O] = \
            res.results[c]["out"].astype(np.float32)
    # V bias is a constant per output column; applied here (exact)
    out += np.asarray(inputs["bv"], dtype=np.float32)[None, None, :]
    return out
